# revision 10
# baseline (speedup 1.0000x reference)
"""Trainium2 8-core GCN kernel (2-layer GCNConv + linear head + softmax).

Strategy (node/row partitioning, dense normalized adjacency):
  - Host: build Ahat = D^-1/2 (A+I) D^-1/2 as a dense fp8-e4m3 matrix, padded
    from 10000 to 10240 nodes; core k owns node rows [k*1280, (k+1)*1280).
  - Device, per core k (all matmuls fp8-e4m3 DoubleRow, fp32 accumulate):
      t1     = x @ W1 for ALL nodes (replicated GEMM; cheaper than the
               all-gather + reload stall it replaces)
      h1T_k  = relu(t1^T Ahat^T[:,k] + b1)    (transposed SpMM -> [512,1280])
      t2_k   = (h1T_k)^T @ W2                 (h1T is directly the lhsT)
      t2     = AllGather(t2_k) in 5 x 1MB chunks
      h2T_k  = relu(t2^T Ahat^T[:,k] + b2)
      out_k  = softmax(h2T_k^T @ Wout + bout) ([1280, 16] f32)
  - Host: concatenate core outputs, trim padding to [10000, 16].

Schedule (the version-2 rewrite for AllGather overlap):
  - Layer-1 SpMM runs its single-chunk column pass (cols 512:1024 ->
    t2 row-tiles m4..7) FIRST, so the first AllGather chunk launches right
    after GEMM1+34us instead of after the full SpMM;  the 5.2MB of t2
    wire time then hides almost entirely under the remaining SpMM work.
  - t2 reload DMAs (gated on AllGather completion) live on the VECTOR DMA
    queue so they cannot head-of-line-block the sync queue's ag_in staging
    writes (which feed the later AllGather chunks).
  - A dummy 64KB AllGather fires at kernel start to absorb the ~11us
    first-collective mesh-begin latency while GEMM1 runs.
  - SpMM2 consumes j-pairs in chunk-arrival order; the softmax head for
    row-tiles m0..3,8,9 runs concurrently with SpMM2's second column pass.
  - Pass-A adjacency columns (0:512 ++ 1024:1280) are host-concatenated
    into one contiguous [128,2,768] tile per j-pair (1.5KB/partition DMA
    descriptors); the pass-B adjacency (13 tiles' worth, 40KB/partition)
    stays resident in SBUF across both layers.

The transposed SpMM (z^T = t^T A^T instead of z = A t) makes each layer's
activation land in [feature, node] layout, which is exactly the lhsT the
following GEMM needs -- no on-device transposes anywhere.  All matmuls use
perf_mode=DoubleRow (256 contraction rows per matmul): lhsT/rhs are
[128, 2, free] pair tiles, element [p, q] = contraction row q*128+p.
"""

import contextlib
import ctypes
import sys
import types

import ml_dtypes
import numpy as np

import concourse.bass as bass
import concourse.mybir as mybir
import concourse.tile as tile
from concourse.bass_utils import run_bass_kernel_spmd

BF16 = ml_dtypes.bfloat16
FP8 = ml_dtypes.float8_e4m3

N_CORES = 8
N_NODES = 10000
F_IN = 512
F_HID = 512
N_CLASSES = 16
NP = 10240            # padded node count (80 * 128)
R = NP // N_CORES     # 1280 rows per core
P = 128
NJ = NP // P          # 80 contraction chunks
NJP = NJ // 2         # 40 DoubleRow contraction pairs
NM = R // P           # 10 row tiles per core
NF = F_HID // P       # 4 feature tiles
NFP = NF // 2         # 2 feature pairs
WA = 768              # sweep-1 adjacency width (node cols 512:1280, contiguous)

# SpMM column sweeps; chunk tuples: (hT column offset, width, at tile off).
# Both layers: sweep1 = cols {512:1024, 1024:1280} (streamed ATs1), then
# sweep2 = cols {0:512} (resident ATs2).  Running sweep1 FIRST in layer 1
# finishes 3MB of t2 rows (m4..9) in one go, so the AllGather wire starts
# early and never goes idle (idle-restart costs ~10us per collective).
S1CH = [(512, 512, 0), (1024, 256, 512)]
S2CH = [(0, 512, 0)]
# AllGather chunks: (m-tile base, n m-tiles).  Launch order = availability:
# m4..7 (2MB) + m89 after sweep1; m01/m23 after sweep2.
AG_CHUNKS = [(4, 4), (8, 2), (0, 2), (2, 2)]

_NTFF_HOOK_INSTALLED = False


def install_ntff_hook():
    """bass_utils' trace=True path wants antenv.axon_hooks; this container
    doesn't ship it, so provide the same ctypes hook trn_boot would."""
    global _NTFF_HOOK_INSTALLED
    if _NTFF_HOOK_INSTALLED:
        return
    _NTFF_HOOK_INSTALLED = True
    try:
        lib = ctypes.CDLL("/opt/axon/libaxon_pjrt.so")
        if not hasattr(lib, "axon_start_nrt_profile"):
            return
    except OSError:
        return
    lib.axon_start_nrt_profile.argtypes = [
        ctypes.POINTER(ctypes.c_int64),
        ctypes.c_size_t,
    ]
    lib.axon_start_nrt_profile.restype = ctypes.c_int64
    lib.axon_stop_nrt_profile.argtypes = [ctypes.c_char_p]
    lib.axon_stop_nrt_profile.restype = ctypes.c_int64

    @contextlib.contextmanager
    def _hook(output_dir, device_ids):
        import jax

        jax.devices()
        if device_ids:
            ids = (ctypes.c_int64 * len(device_ids))(*device_ids)
            rc = lib.axon_start_nrt_profile(ids, len(device_ids))
        else:
            rc = lib.axon_start_nrt_profile(None, 0)
        if rc != 0:
            raise RuntimeError(f"axon_start_nrt_profile rc={rc}")
        try:
            yield
        finally:
            n = lib.axon_stop_nrt_profile(str(output_dir).encode())
            print(f"ntff profile: {n} file(s) -> {output_dir}", file=sys.stderr)

    import antenv

    mod = types.ModuleType("antenv.axon_hooks")
    mod.get_axon_ntff_profile_hook = lambda: _hook
    mod.set_axon_ntff_profile_hook = lambda h: None
    sys.modules["antenv.axon_hooks"] = mod
    antenv.axon_hooks = mod


def split_drain_waits(nc):
    """This walrus build allows only ONE sync-wait per lowered instruction
    (CTRL and pseudo-DMA structs assert on more).  Tile's wait-assignment can
    attach several; keep the last wait on the instruction and move the rest
    onto preceding single-wait NoOps on the same engine stream (waits are
    monotonic >= conditions, so enforcing them earlier in program order on
    the same engine is equivalent)."""
    for f in nc.m.functions:
        for bb in f.blocks:
            insts = bb.instructions
            i = 0
            while i < len(insts):
                inst = insts[i]
                si = getattr(inst, "sync_info", None)
                if si is not None and si.on_wait and len(si.on_wait) > 1:
                    waits = list(si.on_wait)
                    si.on_wait = [waits[-1]]
                    for j, w in enumerate(waits[:-1]):
                        pre = mybir.InstNoOp(
                            name=f"{inst.name}-presync-{j}",
                            engine=inst.engine,
                            ins=[],
                            outs=[],
                            sync_info=mybir.SyncInfo(on_wait=[w], on_update=[]),
                        )
                        insts.insert(i + j, pre)
                        nc.register_instruction(pre, overwrite=True)
                    i += len(waits) - 1
                i += 1


def build_gcn(nc):
    """Emit the SPMD GCN program (identical on every core; per-core data)."""
    f32 = mybir.dt.float32
    bf16 = mybir.dt.bfloat16
    fp8 = mybir.dt.float8e4
    rg = [list(range(N_CORES))]

    # I/O (per-core shards; same names on every core)
    # xTp8[jp, p, jj, cq, m] = x[(2jp+jj)*128+m, cq*128+p]  (fp8 pairs)
    xTp8 = nc.declare_dram_parameter("xTp8", [NJP, P, 2, NF, P], fp8, isOutput=False)
    # ATs1[jp, p, q, c] = AhatT[jp*256+q*128+p, kR + 512 + c]   (cols 512:1280)
    ATs1 = nc.declare_dram_parameter("ATs1", [NJP, P, 2, WA], fp8, isOutput=False)
    # ATs2[jp, p, q, c] = AhatT[jp*256+q*128+p, kR + c]          (cols 0:512)
    ATs2 = nc.declare_dram_parameter("ATs2", [NJP, P, 2, F_HID], fp8, isOutput=False)
    # W pair layouts: W*p8[t, p, q, n] = W[(2t+q)*128 + p, n]
    W1p = nc.declare_dram_parameter("W1p", [NFP, P, 2, F_HID], fp8, isOutput=False)
    W2p = nc.declare_dram_parameter("W2p", [NFP, P, 2, F_HID], fp8, isOutput=False)
    Woutp = nc.declare_dram_parameter("Woutp", [NFP, P, 2, N_CLASSES], fp8, isOutput=False)
    bcols = nc.declare_dram_parameter("bcols", [P, 2 * NF], f32, isOutput=False)
    bout = nc.declare_dram_parameter("bout", [1, N_CLASSES], bf16, isOutput=False)
    # out[p, m*16+c] = prob(node m*128+p, class c); host re-interleaves.
    out = nc.declare_dram_parameter("out", [P, NM * N_CLASSES], f32, isOutput=True)

    # layer-2 collective bounce buffers (internal DRAM), 5 x 1MB chunks, fp8
    ag_in = nc.dram_tensor("ag_in", [R, F_HID], fp8)
    ag_out = [
        nc.dram_tensor(
            f"ag_out{c}", [N_CORES * nt * P, F_HID], fp8, addr_space="Shared"
        )
        for c, (_b, nt) in enumerate(AG_CHUNKS)
    ]
    # warm-up collective buffers (128KB of uninitialized garbage: primes the
    # ncfw/mesh pipeline under GEMM1 so the first real chunk starts warm)
    AGW = 2 * P
    agw_in = nc.dram_tensor("agw_in", [AGW, F_HID], fp8)
    agw_out = nc.dram_tensor("agw_out", [N_CORES * AGW, F_HID], fp8, addr_space="Shared")

    with tile.TileContext(nc) as tc:
        with (
            tc.tile_pool(name="const", bufs=1) as cpool,
            tc.tile_pool(name="tfull", bufs=1) as tpool,
            tc.tile_pool(name="hT", bufs=1) as hpool,
            tc.tile_pool(name="atB", bufs=1) as bpool,
            tc.tile_pool(name="atA", bufs=1) as apool,
            tc.tile_pool(name="work", bufs=6) as wpool,
            tc.tile_pool(name="evac", bufs=4) as epool,
            tc.tile_pool(name="sm", bufs=4) as spool,
            tc.tile_pool(name="psum", bufs=1, space="PSUM") as ppool,
        ):
            # ---- collective warm-up: absorb mesh-begin latency under GEMM1
            nc.gpsimd.collective_compute(
                "AllGather",
                mybir.AluOpType.bypass,
                replica_groups=rg,
                ins=[agw_in[:, :].opt()],
                outs=[agw_out[:, :].opt()],
            )

            # ---- GEMM1 constants (needed immediately) ----
            W1_sb = [cpool.tile([P, 2, F_HID], fp8, tag=f"W1{t}", name=f"W1{t}") for t in range(NFP)]
            for t in range(NFP):
                nc.sync.dma_start(out=W1_sb[t][:], in_=W1p[t, :, :, :])

            # sweep-2 adjacency (cols 0:512): resident in SBUF, both layers
            ats2_sb = [
                bpool.tile([P, 2, F_HID], fp8, tag=f"at2_{jp}", name=f"at2_{jp}")
                for jp in range(NJP)
            ]
            for jp in range(NJP):
                nc.scalar.dma_start(out=ats2_sb[jp][:], in_=ATs2[jp, :, :, :])

            # persistent activation tiles: j-PAIR tiles for DoubleRow,
            # double-buffered across layers (set 0 = t1, set 1 = t2)
            t_pair = [
                [
                    tpool.tile([P, 2, F_HID], fp8, tag=f"tp{s}_{jp}", name=f"tp{s}_{jp}")
                    for jp in range(NJP)
                ]
                for s in range(2)
            ]
            # hT as fp8 feature-pair tiles: hp[layer][t][p, q, m],
            # f-tile index ft = 2t+q  (directly the next GEMM's lhsT pairs)
            hp = [
                [hpool.tile([P, 2, R], fp8, tag=f"h{la}p{t}", name=f"h{la}p{t}") for t in range(NFP)]
                for la in range(2)
            ]

            # ---- layer 1: replicated GEMM1 (fp8 DoubleRow) ----
            for jp in range(NJP):
                xt = wpool.tile([P, 2, NF, P], fp8, tag="xtt", name="xtt")
                nc.sync.dma_start(out=xt[:], in_=xTp8[jp, :, :, :, :])
                for jj in range(2):
                    j = 2 * jp + jj
                    ps = ppool.tile([P, F_HID], f32, tag=f"sp{j % 4}", name=f"g1ps{j % 4}")
                    for t in range(NFP):
                        nc.tensor.matmul(
                            out=ps[:],
                            lhsT=xt[:, jj, 2 * t:2 * t + 2, :],
                            rhs=W1_sb[t][:, :, :],
                            start=(t == 0),
                            stop=(t == NFP - 1),
                            perf_mode=mybir.MatmulPerfMode.DoubleRow,
                        )
                    nc.vector.tensor_copy(out=t_pair[0][jp][:, jj, :], in_=ps[:])

            # ---- deferred constants (needed after GEMM1 starts) ----
            W2_sb = [cpool.tile([P, 2, F_HID], fp8, tag=f"W2{t}", name=f"W2{t}") for t in range(NFP)]
            for t in range(NFP):
                nc.sync.dma_start(out=W2_sb[t][:], in_=W2p[t, :, :, :])
            Wout_sb = [cpool.tile([P, 2, N_CLASSES], fp8, tag=f"Wo{t}", name=f"Wo{t}") for t in range(NFP)]
            for t in range(NFP):
                nc.sync.dma_start(out=Wout_sb[t][:], in_=Woutp[t, :, :, :])
            bcols_sb = cpool.tile([P, 2 * NF], f32, tag="bcols", name="bcols")
            nc.sync.dma_start(out=bcols_sb[:], in_=bcols[:, :])
            bout_sb = cpool.tile([1, N_CLASSES], bf16, tag="bout", name="bout")
            nc.sync.dma_start(out=bout_sb[:], in_=bout[:, :])
            ones_sb = cpool.tile([1, P], bf16, tag="ones", name="ones")
            nc.vector.memset(ones_sb[:], 1.0)
            outsb = cpool.tile([P, NM * N_CLASSES], f32, tag="outsb", name="outsb")

            # sweep-1 adjacency (cols 512:1280): streamed JIT per sweep
            # (front-loading it resident costs ~20% on every matmul under
            # the DMA -- SBUF write-port contention; streaming costs ~3%)
            def get_s1(jp):
                at = wpool.tile([P, 2, WA], fp8, tag="at1", name="at1")
                nc.scalar.dma_start(out=at[:], in_=ATs1[jp, :, :, :])
                return at

            def get_s2(jp):
                return ats2_sb[jp]

            def spmm_pass(layer, chunks, jp_order, names, at_get):
                """One fp8 DoubleRow accumulation sweep over the given column
                chunks of hT[layer] = relu(t^T A^T + b)."""
                pstiles = {}
                bank = 0
                for (off, width, _roff) in chunks:
                    for f in range(NF):
                        pstiles[(off, f)] = ppool.tile(
                            [P, width], f32, tag=f"sp{bank}", name=f"{names}_{bank}"
                        )
                        bank += 1
                src = t_pair[layer]
                for idx, jp in enumerate(jp_order):
                    at = at_get(jp)
                    for f in range(NF):
                        for (off, width, roff) in chunks:
                            nc.tensor.matmul(
                                out=pstiles[(off, f)][:],
                                lhsT=src[jp][:, :, f * P:(f + 1) * P],
                                rhs=at[:, :, roff:roff + width],
                                start=(idx == 0),
                                stop=(idx == NJP - 1),
                                perf_mode=mybir.MatmulPerfMode.DoubleRow,
                            )
                # evacuate: relu(psum + b) -> fp8 pair tiles; f-tile ft=2t+q
                for (off, width, _roff) in chunks:
                    for f in range(NF):
                        nc.vector.tensor_scalar(
                            out=hp[layer][f // 2][:, f % 2, off:off + width],
                            in0=pstiles[(off, f)][:],
                            scalar1=bcols_sb[:, layer * NF + f:layer * NF + f + 1],
                            scalar2=0.0,
                            op0=mybir.AluOpType.add,
                            op1=mybir.AluOpType.max,
                        )

            def gemm2_tiles(ms):
                """t2_k rows for m-tiles `ms` staged into ag_in (as fp8)."""
                for m in ms:
                    ps = ppool.tile([P, F_HID], f32, tag=f"sp{4 + m % 4}", name=f"g2ps{m % 4}")
                    for t in range(NFP):
                        nc.tensor.matmul(
                            out=ps[:],
                            lhsT=hp[0][t][:, :, m * P:(m + 1) * P],
                            rhs=W2_sb[t][:, :, :],
                            start=(t == 0),
                            stop=(t == NFP - 1),
                            perf_mode=mybir.MatmulPerfMode.DoubleRow,
                        )
                    ev = epool.tile([P, F_HID], fp8, tag="g2ev", name="g2ev")
                    nc.vector.tensor_copy(out=ev[:], in_=ps[:])
                    # scalar queue: lands right behind the adjacency loads;
                    # the sync queue carries the AG-gated t2 reloads instead.
                    nc.scalar.dma_start(out=ag_in[m * P:(m + 1) * P, :], in_=ev[:])

            def ag_chunk(c):
                base, nt = AG_CHUNKS[c]
                nc.gpsimd.collective_compute(
                    "AllGather",
                    mybir.AluOpType.bypass,
                    replica_groups=rg,
                    ins=[ag_in[base * P:(base + nt) * P, :].opt()],
                    outs=[ag_out[c][:, :].opt()],
                )

            def load_t2_chunk(c):
                # ag_out[c] row r*nt*128+i*128 -> j = r*10 + base + i
                # (sync queue: empty after startup, so the AG-completion gate
                # on these cannot head-of-line-block anything)
                base, nt = AG_CHUNKS[c]
                for r in range(N_CORES):
                    for i in range(nt):
                        j = r * NM + base + i
                        row = (r * nt + i) * P
                        nc.sync.dma_start(
                            out=t_pair[1][j // 2][:, j % 2, :],
                            in_=ag_out[c][row:row + P, :],
                        )

            def head_tiles(ms):
                """logits + softmax for node tiles `ms` -> outsb columns."""
                for m in ms:
                    ps = ppool.tile([P, N_CLASSES], f32, tag=f"sp{4 + m % 4}", name=f"hps{m % 4}")
                    for t in range(NFP):
                        nc.tensor.matmul(
                            out=ps[:],
                            lhsT=hp[1][t][:, :, m * P:(m + 1) * P],
                            rhs=Wout_sb[t][:, :, :],
                            start=(t == 0),
                            stop=False,
                            perf_mode=mybir.MatmulPerfMode.DoubleRow,
                        )
                    nc.tensor.matmul(
                        out=ps[:],
                        lhsT=ones_sb[:, 0:P],
                        rhs=bout_sb[:],
                        start=False,
                        stop=True,
                    )
                    negmax = spool.tile([P, 1], f32, tag="negmax", name="negmax")
                    nc.vector.tensor_reduce(
                        out=negmax[:], in_=ps[:], axis=mybir.AxisListType.X,
                        op=mybir.AluOpType.max, negate=True,
                    )
                    ex = spool.tile([P, N_CLASSES], f32, tag="ex", name="ex")
                    nc.scalar.activation(
                        out=ex[:], in_=ps[:],
                        func=mybir.ActivationFunctionType.Exp,
                        bias=negmax[:, 0:1],
                    )
                    ssum = spool.tile([P, 1], f32, tag="ssum", name="ssum")
                    nc.vector.tensor_reduce(
                        out=ssum[:], in_=ex[:], axis=mybir.AxisListType.X,
                        op=mybir.AluOpType.add,
                    )
                    rinv = spool.tile([P, 1], f32, tag="rinv", name="rinv")
                    nc.vector.reciprocal(out=rinv[:], in_=ssum[:])
                    nc.vector.tensor_scalar_mul(
                        outsb[:, m * N_CLASSES:(m + 1) * N_CLASSES], ex[:], rinv[:, 0:1]
                    )

            natural = list(range(NJP))
            # SpMM2 j-pair order = AllGather chunk arrival order
            order2 = [
                5 * r + base // 2 + ii
                for (base, nt) in AG_CHUNKS
                for r in range(N_CORES)
                for ii in range(nt // 2)
            ]
            assert sorted(order2) == natural

            # ---- layer 1 SpMM; AG chunks launch at earliest availability --
            spmm_pass(0, S1CH, natural, "s1a", get_s1)
            gemm2_tiles([4, 5, 6, 7])
            ag_chunk(0)
            gemm2_tiles([8, 9])
            ag_chunk(1)
            spmm_pass(0, S2CH, natural, "s1b", get_s2)
            gemm2_tiles([0, 1])
            ag_chunk(2)
            gemm2_tiles([2, 3])
            ag_chunk(3)
            for c in range(len(AG_CHUNKS)):
                load_t2_chunk(c)

            # ---- layer 2 SpMM (consume in arrival order) + head ----
            spmm_pass(1, S1CH, order2, "s2a", get_s1)
            head_tiles([4, 5, 6, 7, 8, 9])
            spmm_pass(1, S2CH, order2, "s2b", get_s2)
            head_tiles([0, 1, 2, 3])

            nc.sync.dma_start(out=out[:, :], in_=outsb[:])

    return nc


def build_inputs(x, edge_index, W1, b1, W2, b2, Wout, bout):
    """Host-side graph preprocessing + per-core shard construction."""
    x = np.asarray(x)
    ei = np.asarray(edge_index)
    n = N_NODES
    src = np.concatenate([ei[0], np.arange(n, dtype=np.int64)])
    dst = np.concatenate([ei[1], np.arange(n, dtype=np.int64)])
    deg = np.bincount(dst, minlength=n).astype(np.float32)
    dinv = 1.0 / np.sqrt(deg)
    normv = (dinv[src] * dinv[dst]).astype(np.float32)

    # dense Ahat^T, padded:  AhatT[src, dst] = norm  (duplicate edges sum)
    AhatT = np.zeros((NP, NP), dtype=np.float32)
    np.add.at(AhatT, (src, dst), normv)
    # DoubleRow pair-interleave: blocks[jp, p, q, :] = AhatT[jp*256+q*128+p, :]
    blocks = AhatT.astype(FP8).reshape(NJP, 2, P, NP).transpose(0, 2, 1, 3)

    xp = np.zeros((NP, F_IN), dtype=np.float32)
    xp[:n] = x
    # xTp8[jp, p, jj, cq, m] = x[(2jp+jj)*128+m, cq*128+p]
    xTp8 = np.ascontiguousarray(
        xp.reshape(NJP, 2, P, NF, P).transpose(0, 4, 1, 3, 2)
    ).astype(FP8)

    def wpairs(W):
        W = np.asarray(W, np.float32)
        # [t, p, q, n] = W[(2t+q)*128+p, n]
        return np.ascontiguousarray(
            W.reshape(NFP, 2, P, W.shape[1]).transpose(0, 2, 1, 3)
        ).astype(FP8)

    W1b = wpairs(W1)
    W2b = wpairs(W2)
    Woutb = wpairs(Wout)
    boutb = np.asarray(bout).reshape(1, N_CLASSES).astype(BF16)
    # biases as per-partition columns: bcols[:, l*NF + f] = b_l[f*128:(f+1)*128]
    bcols = np.stack(
        [np.asarray(b1).reshape(NF, P), np.asarray(b2).reshape(NF, P)], 0
    ).reshape(2 * NF, P).T.astype(np.float32)
    bcols = np.ascontiguousarray(bcols)

    in_maps = []
    for k in range(N_CORES):
        blk = blocks[:, :, :, k * R:(k + 1) * R]
        ATs1 = np.ascontiguousarray(blk[..., 512:1280])
        ATs2 = np.ascontiguousarray(blk[..., 0:512])
        in_maps.append({
            "xTp8": xTp8,
            "ATs1": ATs1,
            "ATs2": ATs2,
            "W1p": W1b,
            "W2p": W2b,
            "Woutp": Woutb,
            "bcols": bcols,
            "bout": boutb,
        })
    return in_maps


_CACHED = {}


def _get_program():
    if "nc" not in _CACHED:
        nc = bass.Bass(num_devices=N_CORES)
        build_gcn(nc)
        split_drain_waits(nc)
        _CACHED["nc"] = nc
    return _CACHED["nc"]


def kernel(x, edge_index, W1, b1, W2, b2, Wout, bout, trace=False):
    install_ntff_hook()
    nc = _get_program()
    in_maps = build_inputs(x, edge_index, W1, b1, W2, b2, Wout, bout)
    res = run_bass_kernel_spmd(
        nc, in_maps, core_ids=list(range(N_CORES)), trace=trace
    )
    # out[p, m*16+c] -> rows m*128+p
    outs = []
    for k in range(N_CORES):
        o = res.results[k]["out"]
        outs.append(o.reshape(P, NM, N_CLASSES).transpose(1, 0, 2).reshape(R, N_CLASSES))
    out = np.concatenate(outs, 0)
    kernel.last_exec_time_ns = res.exec_time_ns
    kernel.last_results = res
    return out[:N_NODES].astype(np.float32)


kernel.last_exec_time_ns = None
kernel.last_results = None
